# revision 19
# baseline (speedup 1.0000x reference)
"""Multi-head attention (B=2, S=4096, H=8, d_head=16) on 8 Trainium2 cores.

Sharding: core -> (batch b = core//4, query quarter of 1024). Each core
computes all 8 heads for its 1024 queries against the compacted valid
keys (~50% of 4096, from seq_mask) of its batch.

Design: PE-array tiling makes the matmuls run concurrently (4-way
bursts), so the kernel is bound by PSUM-evacuation (the exp of the
logits), which is split across the two engines that can read PSUM.

  superstep ss = (q-half qh, chunk c, head-group hg, subpair sp):
  2 heads x 128 keys x 512 queries. The two supersteps of a (qh,c,hg)
  pair are emitted as one PE mode phase: 4 QK MMs (4 row bands, 4
  distinct PSUM banks) run concurrently, then 4 PV MMs of older
  supersteps (concurrent row tiles sharing a PSUM bank hang the
  device -- col tiles may share).
    QK: 2 row-tiled MMs (K=32 bands at partitions 32b, b=2sp+j; the two
        head-groups share the kt bands, rows 0-15 = group 0, rows 16-31
        = group 1, with the *other* group's qt rows zeroed so the K=32
        contraction only picks up the active head) -> run concurrently
        in the PE array (tile_position rows 64sp/64sp+32) into ONE
        lt [128, 1024] tile, head j at cols 512j = its own PSUM bank
        (concurrent row tiles must hit distinct banks).
    exp: one [128,1024] instruction per superstep, alternating engines
        by sp: sp=0 -> ACT true Exp; sp=1 -> DVE Schraudolph
        (int16(A*x+B) = bf16 bits of e^x, ~1.9% sawtooth err that
        partially cancels in softmax; 50% of weights approximated ->
        rel err ~1.3e-2, under the 2e-2 gate). Each engine then works
        every other superstep; with lt bufs=3 the PSUM-slot reuse chain
        exp(ss) -> QK(ss+3) -> exp(ss+3) spans 3 supersteps, so neither
        engine waits on the (possibly cold) PE matmul latency.
    PV: 2 col-tiled MMs (M=17: mask row = denominator + 16 V rows,
        tile_position cols 32*(2sp+j); col tiles may share a bank)
        accumulate into acc[qh,hg] [128,512] (1 bank) over chunks.
  PSUM: 3 x lt [128,1024] + 2 x acc [128,512] = 16KB/partition exactly.
  leftover keys (nv mod 128 <= 16): block-diagonal kf/qf/vf path (one
        full-mode QK over all 8 heads + one row-tiled PV per (qh,hg)).
  out:  raw numerators + denominator rows DMA out; softmax division on
        the host.

The learned scalar bias `b` cancels in softmax (shift invariance) and
max-subtraction is skipped (logits ~ N(0,1); exp cannot overflow fp32).
"""

import sys

import numpy as np

if "/opt/trn_rl_repo" not in sys.path:
    sys.path.insert(0, "/opt/trn_rl_repo")

import ml_dtypes

UNITS = 128
H = 8
DH = 16
B = 2
S = 4096
QPC = 1024  # queries per core (B*S / 8 cores)
QW = 512    # query window per superstep (q-half)
NQ = QPC // QW
VW = 17     # V_aug width: mask at 0 (denominator row), V at 1..16

# Schraudolph exp2-to-bf16-bits constants: bits = int16(A*x + B) with
# A = 128*log2(e), B = 128*127 - C; C ~ 7.33 zeros the mean log-linear
# interpolation error (constant bias cancels in softmax anyway).
SCH_A = 128.0 * 1.4426950408889634
SCH_B = 128.0 * 127.0 - 7.33

TRACE = False
TMPDIR = None
LAST = None

_compiled = {}


def _build(NC, PS):
    """NC full key chunks; PS = per-head partition stride of the packed
    leftover-key path (0 = no leftover path)."""
    import concourse.bass as bass
    import concourse.tile as tile
    from concourse import bacc, mybir

    f32 = mybir.dt.float32
    bf16 = mybir.dt.bfloat16

    nc = bacc.Bacc()
    # kt[p, c, key]: band b=p//32, r=p%32: r<16 -> head b dim r (group
    # 0), r>=16 -> head 4+b dim r-16 (group 1).
    kt = nc.dram_tensor("kt", [128, NC, 128], bf16, kind="ExternalInput")
    # qt[g, p, q]: same band layout; rows of the other group zeroed.
    qt = nc.dram_tensor("qt", [2, 128, QPC], bf16, kind="ExternalInput")
    # va[p, c, h*VW+j]: per head col 0 = validity mask, 1..16 = V.
    va = nc.dram_tensor("va", [128, NC, H * VW], bf16, kind="ExternalInput")
    if PS:
        kf = nc.dram_tensor("kf", [128, 8 * PS], bf16, kind="ExternalInput")
        qf = nc.dram_tensor("qf", [128, QPC], bf16, kind="ExternalInput")
        vf = nc.dram_tensor("vf", [8 * PS, 128], bf16, kind="ExternalInput")
    out = nc.dram_tensor("out", [NQ, 2, 128, QW], f32, kind="ExternalOutput")

    LEAD = 4  # PV emission lag in supersteps (even: pops align to pairs)

    with tile.TileContext(nc) as tc:
        with (
            tc.tile_pool(name="const", bufs=1) as cpool,
            tc.tile_pool(name="lt", bufs=3, space="PSUM") as lt_pool,
            tc.tile_pool(name="acc", bufs=2, space="PSUM") as acc_pool,
            tc.tile_pool(name="exp", bufs=8) as exp_pool,
            tc.tile_pool(name="ev", bufs=4) as ev_pool,
        ):
            # per-chunk input tiles so compute starts as soon as chunk 0
            # lands; DMAs issued in consumption order on the two queues
            # that don't carry exp work (sync, gpsimd).
            kt_c = [cpool.tile([128, 128], bf16, name=f"kt{c}")
                    for c in range(NC)]
            va_c = [cpool.tile([128, H * VW], bf16, name=f"va{c}")
                    for c in range(NC)]
            qt_g = [cpool.tile([128, QPC], bf16, name=f"qt{g}")
                    for g in range(2)]
            nc.sync.dma_start(out=qt_g[0][:, 0:QW], in_=qt[0, :, 0:QW])
            nc.gpsimd.dma_start(out=qt_g[1][:, 0:QW], in_=qt[1, :, 0:QW])
            nc.sync.dma_start(out=kt_c[0], in_=kt[:, 0, :])
            nc.gpsimd.dma_start(out=va_c[0], in_=va[:, 0, :])
            nc.sync.dma_start(out=qt_g[0][:, QW:QPC], in_=qt[0, :, QW:QPC])
            nc.gpsimd.dma_start(out=qt_g[1][:, QW:QPC], in_=qt[1, :, QW:QPC])
            if PS:
                kf_sb = cpool.tile([128, 8 * PS], bf16)
                qf_sb = cpool.tile([128, QPC], bf16)
                vf_sb = cpool.tile([8 * PS, 128], bf16)
                nc.sync.dma_start(out=qf_sb, in_=qf[:, :])
                nc.gpsimd.dma_start(out=kf_sb, in_=kf[:, :])
                nc.gpsimd.dma_start(out=vf_sb, in_=vf[:, :])
                ex_t = cpool.tile([8 * PS, QPC], bf16)
            for c in range(1, NC):
                nc.sync.dma_start(out=kt_c[c], in_=kt[:, c, :])
                nc.gpsimd.dma_start(out=va_c[c], in_=va[:, c, :])

            acc_t = {}
            pend = []

            def emit_pv(p):
                ss, qh, c, hg, sp, e = p
                acc = acc_t[(qh, hg)]
                for j in range(2):
                    b = 2 * sp + j
                    h = 4 * hg + b
                    nc.tensor.matmul(
                        acc[32 * b:32 * b + VW, :],
                        lhsT=va_c[c][:, h * VW:(h + 1) * VW],
                        rhs=e[:, QW * j:QW * (j + 1)],
                        start=(c == 0),
                        stop=(c == NC - 1 and not PS),
                        tile_position=(0, 32 * b),
                        skip_group_check=bool(PS),
                    )
                if c == NC - 1 and sp == 1:
                    # close the accumulation with the leftover keys, then
                    # evacuate + ship. This superstep's exp is on DVE
                    # (sp==1), so the copy goes on ACT.
                    if PS:
                        nc.tensor.matmul(
                            acc[:, :],
                            lhsT=vf_sb[4 * PS * hg:4 * PS * (hg + 1), :],
                            rhs=ex_t[4 * PS * hg:4 * PS * (hg + 1),
                                     qh * QW:(qh + 1) * QW],
                            start=False,
                            stop=True,
                            skip_group_check=True,
                        )
                    ev = ev_pool.tile([128, QW], f32, name="ev", tag="ev")
                    nc.scalar.copy(ev, acc[:, :])
                    nc.sync.dma_start(out=out[qh, hg], in_=ev)

            ss = 0
            for qh in range(NQ):
                for c in range(NC):
                    for hg in range(2):
                        # one PE mode phase covers both subpairs: 4 QK
                        # MMs (4 distinct row bands, 4 distinct PSUM
                        # banks across 2 lt tiles) run concurrently,
                        # then both exps, then 4 PV MMs of older
                        # supersteps -- halves the mode switches and
                        # burst-head latencies per superstep.
                        if c == 0:
                            acc_t[(qh, hg)] = acc_pool.tile(
                                [128, QW], f32, name=f"acc{qh}{hg}",
                                tag="acc",
                            )
                        # PE queue order: QK_a, PV(old pair), QK_b --
                        # QK_b's slot-wait (exp of the previous pair)
                        # then never blocks the PV burst behind it.
                        for sp in range(2):
                            lt = lt_pool.tile([128, 2 * QW], f32,
                                              name="lt", tag="lt")
                            for j in range(2):
                                b = 2 * sp + j
                                nc.tensor.matmul(
                                    lt[:, QW * j:QW * (j + 1)],
                                    lhsT=kt_c[c][32 * b:32 * b + 32, :],
                                    rhs=qt_g[hg][32 * b:32 * b + 32,
                                                 qh * QW:(qh + 1) * QW],
                                    start=True,
                                    stop=True,
                                    tile_position=(32 * b, 0),
                                )
                            e = exp_pool.tile([128, 2 * QW], bf16,
                                              name="e", tag="e")
                            if sp == 0:
                                nc.scalar.activation(
                                    e, lt,
                                    mybir.ActivationFunctionType.Exp,
                                )
                            else:
                                nc.vector.tensor_scalar(
                                    e.bitcast(mybir.dt.int16),
                                    lt,
                                    float(SCH_A),
                                    float(SCH_B),
                                    mybir.AluOpType.mult,
                                    mybir.AluOpType.add,
                                )
                            pend.append((ss, qh, c, hg, sp, e))
                            ss += 1
                        while len(pend) > LEAD:
                            emit_pv(pend.pop(0))
                        if PS and ss == 4:
                            # packed leftover keys: one block-diagonal
                            # QK for all 8 heads (contraction = all
                            # 128 Q channels), exp'd once into ex_t.
                            ltx = lt_pool.tile([128, QPC], f32,
                                               name="ltx", tag="lt")
                            for half in range(2):
                                s = half * QW
                                nc.tensor.matmul(
                                    ltx[0:8 * PS, s:s + QW],
                                    lhsT=kf_sb[:, :],
                                    rhs=qf_sb[:, s:s + QW],
                                    start=True,
                                    stop=True,
                                )
                            nc.scalar.activation(
                                ex_t, ltx[0:8 * PS, :],
                                mybir.ActivationFunctionType.Exp,
                            )
            for p in pend:
                emit_pv(p)
    nc.compile()
    return nc


def _get_compiled(NC, PS):
    if (NC, PS) not in _compiled:
        _compiled[(NC, PS)] = _build(NC, PS)
    return _compiled[(NC, PS)]


def kernel(memory, query, seq_mask, b):
    global LAST
    memory = np.asarray(memory, dtype=np.float32)
    query = np.asarray(query, dtype=np.float32)
    seq_mask = np.asarray(seq_mask)
    bf16 = ml_dtypes.bfloat16

    idx = [np.flatnonzero(seq_mask[bb] != 0) for bb in range(B)]
    nv = [len(i) for i in idx]
    nvmax = max(nv)
    n_left = nvmax - (nvmax // 128) * 128
    if 0 < n_left <= 16 and nvmax >= 128:
        # leftover keys go through the packed block-diagonal path
        NC = nvmax // 128
        PS = 8 if n_left <= 8 else 16
    else:
        NC = max(1, (nvmax + 127) // 128)
        PS = 0
    NK = NC * 128

    # band layout: head h -> partitions 32*(h%4) + 16*(h//4) + d
    perm = np.empty(128, np.int64)
    for h in range(H):
        perm[32 * (h % 4) + 16 * (h // 4) + np.arange(DH)] = \
            h * DH + np.arange(DH)

    kts = []
    vas = []
    kfs = []
    vfs = []
    for bb in range(B):
        kpad = np.zeros((NK, UNITS), np.float32)
        kpad[:min(nv[bb], NK)] = memory[bb, :, :UNITS][idx[bb]][:NK]
        vpad = np.zeros((NK, UNITS), np.float32)
        vpad[:min(nv[bb], NK)] = memory[bb, :, UNITS:][idx[bb]][:NK]
        # kt: [128, NC, 128]: partition p = band layout, cols = keys
        ktr = kpad.T[perm].reshape(128, NC, 128)
        kts.append(np.ascontiguousarray(ktr).astype(bf16))
        # va: [128 partitions=keys, NC, H*VW]; per head: col 0 = validity
        # mask (pad keys have K=0 -> logit 0 -> exp 1, but mask 0 removes
        # them from the denominator, V=0 from the numerator), 1..16 = V
        va_arr = np.zeros((NC, 128, H, VW), np.float32)
        va_arr[..., 1:] = vpad.reshape(NC, 128, H, DH)
        valid = (np.arange(NK) < nv[bb]).astype(np.float32)
        va_arr[..., 0] = valid.reshape(NC, 128)[:, :, None]
        va_arr = va_arr.transpose(1, 0, 2, 3).reshape(128, NC, H * VW)
        vas.append(np.ascontiguousarray(va_arr).astype(bf16))
        if PS:
            nl = max(0, nv[bb] - NK)
            klft = memory[bb, :, :UNITS][idx[bb]][NK:]  # [nl, 128]
            vlft = memory[bb, :, UNITS:][idx[bb]][NK:]
            # kf[h*16+d, h*PS+k] = K_h[k, d]  (block diagonal)
            kf_arr = np.zeros((128, 8 * PS), np.float32)
            vf_arr = np.zeros((8 * PS, 128), np.float32)
            for h in range(H):
                for k in range(nl):
                    kf_arr[h * DH:(h + 1) * DH, h * PS + k] = \
                        klft[k, h * DH:(h + 1) * DH]
                    hg, hi = divmod(h, 4)
                    vf_arr[4 * PS * hg + hi * PS + k, 32 * hi] = 1.0
                    vf_arr[4 * PS * hg + hi * PS + k,
                           32 * hi + 1:32 * hi + 1 + DH] = \
                        vlft[k, h * DH:(h + 1) * DH]
            kfs.append(kf_arr.astype(bf16))
            vfs.append(vf_arr.astype(bf16))

    in_maps = []
    for core in range(8):
        bb, qslot = divmod(core, 4)
        q0 = qslot * QPC
        qc = query[bb, q0:q0 + QPC, :] * (DH ** -0.5)  # [1024, 128]
        qtr = qc.T  # [128 channels, 1024]
        # qt[g]: band layout with the other group's rows zeroed
        qt_arr = np.zeros((2, 128, QPC), np.float32)
        for g in range(2):
            for h in range(4 * g, 4 * g + 4):
                rows = 32 * (h % 4) + 16 * g + np.arange(DH)
                qt_arr[g, rows] = qtr[h * DH:(h + 1) * DH]
        im = {
            "kt": kts[bb],
            "qt": np.ascontiguousarray(qt_arr).astype(bf16),
            "va": vas[bb],
        }
        if PS:
            im["kf"] = kfs[bb]
            im["vf"] = vfs[bb]
            im["qf"] = np.ascontiguousarray(qtr).astype(bf16)  # [128, 1024]
        in_maps.append(im)

    nc = _get_compiled(NC, PS)
    from concourse.bass_utils import run_bass_kernel_spmd

    res = run_bass_kernel_spmd(
        nc, in_maps, core_ids=list(range(8)), trace=TRACE, tmpdir=TMPDIR
    )
    LAST = res

    out_full = np.empty((B, S, H * DH), np.float32)
    for core in range(8):
        bb, qslot = divmod(core, 4)
        o = np.asarray(res.results[core]["out"], np.float32)  # [NQ,2,128,QW]
        # rows 32*hi+1 .. 32*hi+16 of block hi hold head (hg*4+hi)'s
        # numerators; row 32*hi is the softmax denominator.
        o = o.reshape(NQ, 2, 4, 32, QW)
        o = o[:, :, :, 1:DH + 1, :] / o[:, :, :, 0:1, :]
        # [qh, hg, hi, d, q] -> [qh, q, hg, hi, d]
        o = o.transpose(0, 4, 1, 2, 3).reshape(QPC, H * DH)
        out_full[bb, qslot * QPC:(qslot + 1) * QPC] = o
    return out_full


# revision 24
# speedup vs baseline: 1.1014x; 1.1014x over previous
"""Multi-head attention (B=2, S=4096, H=8, d_head=16) on 8 Trainium2 cores.

Sharding: core -> (batch b = core//4, query quarter of 1024). Each core
computes all 8 heads for its 1024 queries against the compacted valid
keys (~50% of 4096, from seq_mask) of its batch.

Design: PE-array tiling makes the matmuls run concurrently (4-way
bursts), so the kernel is bound by PSUM-evacuation (the exp of the
logits), which is split across the two engines that can read PSUM.

  superstep ss = (q-half qh, chunk c, head-group hg, subpair sp):
  2 heads x 128 keys x 512 queries. The two supersteps of a (qh,c,hg)
  pair are emitted as one PE mode phase: 4 QK MMs (4 row bands, 4
  distinct PSUM banks) run concurrently, then 4 PV MMs of older
  supersteps (concurrent row tiles sharing a PSUM bank hang the
  device -- col tiles may share).
    QK: 2 row-tiled MMs (K=32 bands at partitions 32b, b=2sp+j; the two
        head-groups share the kt bands, rows 0-15 = group 0, rows 16-31
        = group 1, with the *other* group's qt rows zeroed so the K=32
        contraction only picks up the active head) -> run concurrently
        in the PE array (tile_position rows 64sp/64sp+32) into ONE
        lt [128, 1024] tile, head j at cols 512j = its own PSUM bank
        (concurrent row tiles must hit distinct banks).
    exp: one [128,1024] instruction per superstep, alternating engines
        by sp: sp=0 -> ACT true Exp; sp=1 -> DVE Schraudolph
        (int16(A*x+B) = bf16 bits of e^x, ~1.9% sawtooth err that
        partially cancels in softmax; 50% of weights approximated ->
        rel err ~1.3e-2, under the 2e-2 gate). Each engine then works
        every other superstep; with lt bufs=3 the PSUM-slot reuse chain
        exp(ss) -> QK(ss+3) -> exp(ss+3) spans 3 supersteps, so neither
        engine waits on the (possibly cold) PE matmul latency.
    PV: 2 col-tiled MMs (M=17: mask row = denominator + 16 V rows,
        tile_position cols 32*(2sp+j); col tiles may share a bank)
        accumulate into acc[qh,hg] [128,512] (1 bank) over chunks.
  PSUM: 2 x lta [128,1024] + 3 x ltb [128,512] + 1 x acc [128,512]
        (one live accumulator; hg is an outer loop) = 16KB exactly.
  leftover keys (nv mod 128 <= 16): block-diagonal kf/qf/vf path (one
        full-mode QK over all 8 heads + one row-tiled PV per (qh,hg)).
  out:  raw numerators + denominator rows DMA out; softmax division on
        the host.

The learned scalar bias `b` cancels in softmax (shift invariance) and
max-subtraction is skipped (logits ~ N(0,1); exp cannot overflow fp32).
"""

import sys

import numpy as np

if "/opt/trn_rl_repo" not in sys.path:
    sys.path.insert(0, "/opt/trn_rl_repo")

import ml_dtypes

UNITS = 128
H = 8
DH = 16
B = 2
S = 4096
QPC = 1024  # queries per core (B*S / 8 cores)
QW = 512    # query window per superstep (q-half)
NQ = QPC // QW
VW = 17     # V_aug width: mask at 0 (denominator row), V at 1..16

# Schraudolph exp2-to-bf16-bits constants: bits = int16(A*x + B) with
# A = 128*log2(e), B = 128*127 - C; C ~ 7.33 zeros the mean log-linear
# interpolation error (constant bias cancels in softmax anyway).
SCH_A = 128.0 * 1.4426950408889634
SCH_B = 128.0 * 127.0 - 7.33

TRACE = False
TMPDIR = None
LAST = None

_compiled = {}


def _build(NC, PS):
    """NC full key chunks; PS = per-head partition stride of the packed
    leftover-key path (0 = no leftover path)."""
    import concourse.bass as bass
    import concourse.tile as tile
    from concourse import bacc, mybir

    f32 = mybir.dt.float32
    bf16 = mybir.dt.bfloat16

    nc = bacc.Bacc()
    # kt[p, c, key]: band b=p//32, r=p%32: r<16 -> head b dim r (group
    # 0), r>=16 -> head 4+b dim r-16 (group 1).
    kt = nc.dram_tensor("kt", [128, NC, 128], bf16, kind="ExternalInput")
    # qt[g, p, q]: same band layout; rows of the other group zeroed.
    qt = nc.dram_tensor("qt", [2, 128, QPC], bf16, kind="ExternalInput")
    # va[p, c, h*VW+j]: per head col 0 = validity mask, 1..16 = V.
    va = nc.dram_tensor("va", [128, NC, H * VW], bf16, kind="ExternalInput")
    if PS:
        kf = nc.dram_tensor("kf", [128, 8 * PS], bf16, kind="ExternalInput")
        qf = nc.dram_tensor("qf", [128, QPC], bf16, kind="ExternalInput")
        vf = nc.dram_tensor("vf", [8 * PS, 128], bf16, kind="ExternalInput")
    out = nc.dram_tensor("out", [NQ, 2, 128, QW], f32, kind="ExternalOutput")

    LEAD = 4  # PV emission lag in supersteps (even: pops align to pairs)

    with tile.TileContext(nc) as tc:
        with (
            tc.tile_pool(name="const", bufs=1) as cpool,
            tc.tile_pool(name="lta", bufs=2, space="PSUM") as a_pool,
            tc.tile_pool(name="ltb", bufs=3, space="PSUM") as b_pool,
            tc.tile_pool(name="acc", bufs=1, space="PSUM") as acc_pool,
            tc.tile_pool(name="exp", bufs=8) as exp_pool,
            tc.tile_pool(name="ev", bufs=4) as ev_pool,
        ):
            # per-chunk input tiles so compute starts as soon as chunk 0
            # lands; DMAs issued in consumption order on the two queues
            # that don't carry exp work (sync, gpsimd).
            kt_c = [cpool.tile([128, 128], bf16, name=f"kt{c}")
                    for c in range(NC)]
            va_c = [cpool.tile([128, H * VW], bf16, name=f"va{c}")
                    for c in range(NC)]
            qt_g = [cpool.tile([128, QPC], bf16, name=f"qt{g}")
                    for g in range(2)]
            nc.sync.dma_start(out=qt_g[0][:, 0:QW], in_=qt[0, :, 0:QW])
            nc.gpsimd.dma_start(out=qt_g[1][:, 0:QW], in_=qt[1, :, 0:QW])
            nc.sync.dma_start(out=kt_c[0], in_=kt[:, 0, :])
            nc.gpsimd.dma_start(out=va_c[0], in_=va[:, 0, :])
            nc.sync.dma_start(out=qt_g[0][:, QW:QPC], in_=qt[0, :, QW:QPC])
            nc.gpsimd.dma_start(out=qt_g[1][:, QW:QPC], in_=qt[1, :, QW:QPC])
            if PS:
                kf_sb = cpool.tile([128, 8 * PS], bf16)
                qf_sb = cpool.tile([128, QPC], bf16)
                vf_sb = cpool.tile([8 * PS, 128], bf16)
                nc.sync.dma_start(out=qf_sb, in_=qf[:, :])
                nc.gpsimd.dma_start(out=kf_sb, in_=kf[:, :])
                nc.gpsimd.dma_start(out=vf_sb, in_=vf[:, :])
                ex_t = cpool.tile([8 * PS, QPC], bf16)
            for c in range(1, NC):
                nc.sync.dma_start(out=kt_c[c], in_=kt[:, c, :])
                nc.gpsimd.dma_start(out=va_c[c], in_=va[:, c, :])

            acc_t = {}
            pend = []

            def emit_pv(p):
                ss, qh, c, hg, sp, e = p
                acc = acc_t[(qh, hg)]
                for j in range(2):
                    b = 2 * sp + j
                    h = 4 * hg + b
                    nc.tensor.matmul(
                        acc[32 * b:32 * b + VW, :],
                        lhsT=va_c[c][:, h * VW:(h + 1) * VW],
                        rhs=e[:, QW * j:QW * (j + 1)],
                        start=(c == 0),
                        stop=(c == NC - 1 and not PS),
                        tile_position=(0, 32 * b),
                        skip_group_check=bool(PS),
                    )
                if c == NC - 1 and sp == 1:
                    # close the accumulation with the leftover keys, then
                    # evacuate + ship. This superstep's exp is on DVE
                    # (sp==1), so the copy goes on ACT.
                    if PS:
                        nc.tensor.matmul(
                            acc[:, :],
                            lhsT=vf_sb[4 * PS * hg:4 * PS * (hg + 1), :],
                            rhs=ex_t[4 * PS * hg:4 * PS * (hg + 1),
                                     qh * QW:(qh + 1) * QW],
                            start=False,
                            stop=True,
                            skip_group_check=True,
                        )
                    ev = ev_pool.tile([128, QW], f32, name="ev", tag="ev")
                    nc.scalar.copy(ev, acc[:, :])
                    nc.sync.dma_start(out=out[qh, hg], in_=ev)

            ss = 0
            for qh in range(NQ):
                for hg in range(2):
                    for c in range(NC):
                        # one PE mode phase covers both subpairs: 4 QK
                        # MMs (4 distinct row bands, 4 distinct PSUM
                        # banks across 2 lt tiles) run concurrently,
                        # then both exps, then 4 PV MMs of older
                        # supersteps -- halves the mode switches and
                        # burst-head latencies per superstep.
                        if c == 0:
                            acc_t[(qh, hg)] = acc_pool.tile(
                                [128, QW], f32, name=f"acc{qh}{hg}",
                                tag="acc",
                            )
                        # ACT-side lt: one [128,1024] tile, bufs=2 ->
                        # reuse distance 2 pairs (WAR fully hidden).
                        # DVE-side lt: two 1-bank [128,512] tiles,
                        # bufs=3 -> the distance-1 reuse lands between
                        # the DVE's own two instructions (self-hidden),
                        # so no engine ever waits on a cold matmul.
                        lta = a_pool.tile([128, 2 * QW], f32,
                                          name="lta", tag="lta")
                        ltb = [b_pool.tile([128, QW], f32,
                                           name=f"ltb{j}", tag="ltb")
                               for j in range(2)]
                        for j in range(2):
                            nc.tensor.matmul(
                                lta[:, QW * j:QW * (j + 1)],
                                lhsT=kt_c[c][32 * j:32 * j + 32, :],
                                rhs=qt_g[hg][32 * j:32 * j + 32,
                                             qh * QW:(qh + 1) * QW],
                                start=True,
                                stop=True,
                                tile_position=(32 * j, 0),
                            )
                        for j in range(2):
                            b = 2 + j
                            nc.tensor.matmul(
                                ltb[j][:, :],
                                lhsT=kt_c[c][32 * b:32 * b + 32, :],
                                rhs=qt_g[hg][32 * b:32 * b + 32,
                                             qh * QW:(qh + 1) * QW],
                                start=True,
                                stop=True,
                                tile_position=(32 * b, 0),
                            )
                        ea = exp_pool.tile([128, 2 * QW], bf16,
                                           name="ea", tag="e")
                        nc.scalar.activation(
                            ea, lta, mybir.ActivationFunctionType.Exp,
                        )
                        pend.append((ss, qh, c, hg, 0, ea))
                        ss += 1
                        eb = exp_pool.tile([128, 2 * QW], bf16,
                                           name="eb", tag="e")
                        for j in range(2):
                            nc.vector.tensor_scalar(
                                eb.bitcast(mybir.dt.int16)[
                                    :, QW * j:QW * (j + 1)],
                                ltb[j],
                                float(SCH_A),
                                float(SCH_B),
                                mybir.AluOpType.mult,
                                mybir.AluOpType.add,
                            )
                        pend.append((ss, qh, c, hg, 1, eb))
                        ss += 1
                        while len(pend) > LEAD:
                            emit_pv(pend.pop(0))
                        if PS and ss == 4:
                            # packed leftover keys: one block-diagonal
                            # QK for all 8 heads (contraction = all
                            # 128 Q channels), exp'd once into ex_t.
                            ltx = a_pool.tile([128, QPC], f32,
                                              name="ltx", tag="lta")
                            for half in range(2):
                                s = half * QW
                                nc.tensor.matmul(
                                    ltx[0:8 * PS, s:s + QW],
                                    lhsT=kf_sb[:, :],
                                    rhs=qf_sb[:, s:s + QW],
                                    start=True,
                                    stop=True,
                                )
                            nc.scalar.activation(
                                ex_t, ltx[0:8 * PS, :],
                                mybir.ActivationFunctionType.Exp,
                            )
            for p in pend:
                emit_pv(p)
    nc.compile()
    return nc


def _get_compiled(NC, PS):
    if (NC, PS) not in _compiled:
        _compiled[(NC, PS)] = _build(NC, PS)
    return _compiled[(NC, PS)]


def kernel(memory, query, seq_mask, b):
    global LAST
    memory = np.asarray(memory, dtype=np.float32)
    query = np.asarray(query, dtype=np.float32)
    seq_mask = np.asarray(seq_mask)
    bf16 = ml_dtypes.bfloat16

    idx = [np.flatnonzero(seq_mask[bb] != 0) for bb in range(B)]
    nv = [len(i) for i in idx]
    nvmax = max(nv)
    n_left = nvmax - (nvmax // 128) * 128
    if 0 < n_left <= 16 and nvmax >= 128:
        # leftover keys go through the packed block-diagonal path
        NC = nvmax // 128
        PS = 8 if n_left <= 8 else 16
    else:
        NC = max(1, (nvmax + 127) // 128)
        PS = 0
    NK = NC * 128

    # band layout: head h -> partitions 32*(h%4) + 16*(h//4) + d
    perm = np.empty(128, np.int64)
    for h in range(H):
        perm[32 * (h % 4) + 16 * (h // 4) + np.arange(DH)] = \
            h * DH + np.arange(DH)

    kts = []
    vas = []
    kfs = []
    vfs = []
    for bb in range(B):
        kpad = np.zeros((NK, UNITS), np.float32)
        kpad[:min(nv[bb], NK)] = memory[bb, :, :UNITS][idx[bb]][:NK]
        vpad = np.zeros((NK, UNITS), np.float32)
        vpad[:min(nv[bb], NK)] = memory[bb, :, UNITS:][idx[bb]][:NK]
        # kt: [128, NC, 128]: partition p = band layout, cols = keys
        ktr = kpad.T[perm].reshape(128, NC, 128)
        kts.append(np.ascontiguousarray(ktr).astype(bf16))
        # va: [128 partitions=keys, NC, H*VW]; per head: col 0 = validity
        # mask (pad keys have K=0 -> logit 0 -> exp 1, but mask 0 removes
        # them from the denominator, V=0 from the numerator), 1..16 = V
        va_arr = np.zeros((NC, 128, H, VW), np.float32)
        va_arr[..., 1:] = vpad.reshape(NC, 128, H, DH)
        valid = (np.arange(NK) < nv[bb]).astype(np.float32)
        va_arr[..., 0] = valid.reshape(NC, 128)[:, :, None]
        va_arr = va_arr.transpose(1, 0, 2, 3).reshape(128, NC, H * VW)
        vas.append(np.ascontiguousarray(va_arr).astype(bf16))
        if PS:
            nl = max(0, nv[bb] - NK)
            klft = memory[bb, :, :UNITS][idx[bb]][NK:]  # [nl, 128]
            vlft = memory[bb, :, UNITS:][idx[bb]][NK:]
            # kf[h*16+d, h*PS+k] = K_h[k, d]  (block diagonal)
            kf_arr = np.zeros((128, 8 * PS), np.float32)
            vf_arr = np.zeros((8 * PS, 128), np.float32)
            for h in range(H):
                for k in range(nl):
                    kf_arr[h * DH:(h + 1) * DH, h * PS + k] = \
                        klft[k, h * DH:(h + 1) * DH]
                    hg, hi = divmod(h, 4)
                    vf_arr[4 * PS * hg + hi * PS + k, 32 * hi] = 1.0
                    vf_arr[4 * PS * hg + hi * PS + k,
                           32 * hi + 1:32 * hi + 1 + DH] = \
                        vlft[k, h * DH:(h + 1) * DH]
            kfs.append(kf_arr.astype(bf16))
            vfs.append(vf_arr.astype(bf16))

    in_maps = []
    for core in range(8):
        bb, qslot = divmod(core, 4)
        q0 = qslot * QPC
        qc = query[bb, q0:q0 + QPC, :] * (DH ** -0.5)  # [1024, 128]
        qtr = qc.T  # [128 channels, 1024]
        # qt[g]: band layout with the other group's rows zeroed
        qt_arr = np.zeros((2, 128, QPC), np.float32)
        for g in range(2):
            for h in range(4 * g, 4 * g + 4):
                rows = 32 * (h % 4) + 16 * g + np.arange(DH)
                qt_arr[g, rows] = qtr[h * DH:(h + 1) * DH]
        im = {
            "kt": kts[bb],
            "qt": np.ascontiguousarray(qt_arr).astype(bf16),
            "va": vas[bb],
        }
        if PS:
            im["kf"] = kfs[bb]
            im["vf"] = vfs[bb]
            im["qf"] = np.ascontiguousarray(qtr).astype(bf16)  # [128, 1024]
        in_maps.append(im)

    nc = _get_compiled(NC, PS)
    from concourse.bass_utils import run_bass_kernel_spmd

    res = run_bass_kernel_spmd(
        nc, in_maps, core_ids=list(range(8)), trace=TRACE, tmpdir=TMPDIR
    )
    LAST = res

    out_full = np.empty((B, S, H * DH), np.float32)
    for core in range(8):
        bb, qslot = divmod(core, 4)
        o = np.asarray(res.results[core]["out"], np.float32)  # [NQ,2,128,QW]
        # rows 32*hi+1 .. 32*hi+16 of block hi hold head (hg*4+hi)'s
        # numerators; row 32*hi is the softmax denominator.
        o = o.reshape(NQ, 2, 4, 32, QW)
        o = o[:, :, :, 1:DH + 1, :] / o[:, :, :, 0:1, :]
        # [qh, hg, hi, d, q] -> [qh, q, hg, hi, d]
        o = o.transpose(0, 4, 1, 2, 3).reshape(QPC, H * DH)
        out_full[bb, qslot * QPC:(qslot + 1) * QPC] = o
    return out_full
